# revision 27
# baseline (speedup 1.0000x reference)
"""Trainium2 Bass kernel for nn_ConvexMLPBlock.

Reference computation (B=64, HW=196, D=768, E=256, C=10):
    S[b,h,e]  = (x[b,h,:] @ ag_w[e,:] + ag_b[e]) > 0          (sign patterns)
    z[b,h,p]  = x[b,h,:] @ lm_w[p,:]        (p = e*C + c)
    preds[b,c] = sum_{h,e} S[b,h,e] * z[b,h,e,c] / (HW*E)

Restructured to avoid materializing z (49 GFLOP -> ~10 GFLOP):
    G_b[e,d]   = sum_h S[b,h,e] * x[b,h,d]                    (per-batch masked moment)
    preds[b,c] = (1/(HW*E)) * sum_{e,d} G_b[e,d] * W[e,c,d]   (W = lm_w.reshape(E,C,D))

Sharding: data-parallel over B across the 8 NeuronCores (8 batches/core);
host concatenates the per-core (8,10) outputs.

Per-core pipeline:
    mm1: S^T[e,t] over all local tokens t=(b,h). Split x = x_hi(fp16) + x_lo;
         the hi*hi term runs as 6 fp16 matmuls (K=128 each); the two cross
         terms hi*lo + lo*hi run as 6 fp8 DoubleRow matmuls (each contracts
         K=128 over 2 pair slots: slot0 = w_hi8*x_lo8, slot1 = w_lo8*x_hi8,
         operands pre-scaled by 2^+-5.5 so each product has exact scale 1 and
         both fp8 operands sit in e4m3's normal range). Sign error: 14 flips
         out of 3.2M (CPU-simulated), rel err 2.3e-3 total.
    threshold: DVE tensor_scalar (psum + bias) > 0 -> 1.0/0.0 (bf16, exact)
    PE-transpose S^T -> S natural (h on partitions) per batch (bf16)
    mm2: G^T_b[d,e] contraction over h, fp16 (S exact in fp16; only x
         rounded). xn is zero-padded to 128 rows per (b, ht) piece so both
         pieces run K=128 (pad rows of sn are memset to 0 once).
    final: all 96 cross-product matmuls (dt, e-group) accumulate into ONE
           [128, C*EG] PSUM tile (the diagonal mask commutes with the sum);
           one DVE mask-mult + one selection matmul + reduce finish it.

DMA: ~17 large contiguous transfers (vs ~60 small) over the 3 DMA queues
(SP / Activation / gpsimd). The fabric is globally packet- and byte-limited
(~100 pkts/us, ~130 GB/s per queue), so each queue carries its transfers in
strict consumption order (xt -> xn -> wfin) and the late bulk loads (wfin)
are dep-gated behind mm1 thresholds to keep their packets off the fabric
while the xt stream feeds mm1. One SBUF tile per transfer (a tile with two
writers serializes them and makes every reader wait for both).
"""

import numpy as np

import concourse.bass as bass
import concourse.mybir as mybir
import concourse.tile as tile
from concourse.tile import add_dep_helper
from concourse.bass_utils import run_bass_kernel_spmd

# Problem constants (hardcoded per contract).
B = 64
HW = 196
D = 768
E = 256
C = 10
NCORES = 8
BL = B // NCORES          # local batches per core
T = BL * HW               # local tokens = 1568
KT = D // 128             # 6 d-tiles
ET = E // 128             # 2 e-tiles
W1 = 392                  # mm1 moving-dim chunk (4 chunks of 392 = 1568)
NCH = T // W1
EG = 16                   # e's per final-stage group
NG = E // EG              # 16 groups

FP32 = mybir.dt.float32
BF16 = mybir.dt.bfloat16
FP16 = mybir.dt.float16
FP8 = mybir.dt.float8e4

S_HI = 2.0 ** -5.5        # fp8 pre-scale on hi components
S_LO = 2.0 ** 5.5         # fp8 pre-scale on lo components

NWARM = 34                # PE warm-up matmuls bridging the DMA prologue


def _patched_drain_and_barrier(self, tick_clock, wait_clock):
    """This toolchain's walrus rejects >1 sync-wait on CTRL-class (Drain)
    instructions. Split the tail drain's global-clock waits across multiple
    single-wait drains. Semantics preserved: SP observes every DMA-queue
    semaphore before the all-engine barrier."""
    drain_inst = self.nc.sync.drain()
    wait_clock.add_sem_waits(
        drain_inst.ins, tile.ScopedClock({None: tick_clock.global_clock})
    )
    si = drain_inst.ins.sync_info
    if si is not None and si.on_wait is not None and len(si.on_wait) > 1:
        waits = list(si.on_wait)
        drain_inst.ins.sync_info = mybir.SyncInfo(
            on_wait=[waits[0]], on_update=list(si.on_update or [])
        )
        for w in waits[1:]:
            extra = self.nc.sync.drain()
            extra.ins.sync_info = mybir.SyncInfo(on_wait=[w], on_update=[])

    self.nc.all_engine_barrier()
    assert self.sems is not None
    popped = self.nc._tile_sem_poison_stack.pop()
    assert popped is self._sem_poison
    self.nc.clear_and_free_semaphores(list(self.sems.allocated().values()))
    self.nc.all_engine_barrier()


tile.TileContext._drain_and_barrier = _patched_drain_and_barrier


def _split_multiwait_json(bj: bytes) -> bytes:
    """Walrus in this toolchain accepts at most one sync-wait per instruction.
    For any instruction with N>1 waits, hoist N-1 waits onto same-engine NoOps
    inserted immediately before it. Engines execute program-order, so for
    compute instructions this is semantically identical; for DMAs it
    conservatively blocks the issuing engine instead of the queue."""
    import json

    m = json.loads(bj)
    changed = False
    for fn in m["functions"]:
        for bb in fn["blocks"]:
            new_insts = []
            for inst in bb["instructions"]:
                si = inst.get("sync_info")
                ow = (si or {}).get("on_wait") or []
                if len(ow) > 1:
                    for j, w in enumerate(ow[:-1]):
                        new_insts.append(
                            {
                                "name": f"{inst['name']}__w{j}",
                                "opcode": "NoOp",
                                "engine": inst["engine"],
                                "ins": [],
                                "outs": [],
                                "sync_info": {"on_update": [], "on_wait": [w]},
                            }
                        )
                    si["on_wait"] = [ow[-1]]
                    changed = True
                new_insts.append(inst)
            bb["instructions"] = new_insts
    if not changed:
        return bj
    return json.dumps(m).encode()


_orig_to_json_bytes = bass.Bass.to_json_bytes


def _patched_to_json_bytes(self, *a, **k):
    return _split_multiwait_json(_orig_to_json_bytes(self, *a, **k))


bass.Bass.to_json_bytes = _patched_to_json_bytes


def build_program():
    DR = mybir.MatmulPerfMode.DoubleRow

    nc = bass.Bass()

    # host layouts put the SBUF partition dim first so each load is ONE
    # contiguous-iteration DMA (dst/src element orders match).
    # xt layouts are nch-major so each column-chunk DMA is one contiguous
    # 4704B run per partition (descriptor generation cost scales with the
    # number of non-contiguous runs; short strided rows cost ~12us per issue)
    xt_hi_d = nc.dram_tensor("xt_hi", (128, NCH, KT, W1), FP16,
                             kind="ExternalInput").ap()
    # xt8 carries only the x_lo8 slot; the x_hi8 slot is synthesized
    # on-chip by a scaled DVE cast of xt_hi (same rounding as the host path),
    # saving 1.18MB on the bandwidth-capped scalar queue
    xt8_d = nc.dram_tensor("xt8", (128, NCH, KT, W1), FP8,
                           kind="ExternalInput").ap()
    agt_hi_d = nc.dram_tensor("agt_hi", (128, KT, E), FP16, kind="ExternalInput").ap()
    agt8_d = nc.dram_tensor("agt8", (128, KT, 2, E), FP8, kind="ExternalInput").ap()
    agb_d = nc.dram_tensor("agb", (128, ET), FP32, kind="ExternalInput").ap()
    # xn_a[r, b, d] = x[b, r, d]; xn_b[r, b, d] = x[b, 128+r, d]
    xn_a_d = nc.dram_tensor("xn_a", (128, BL, D), FP16, kind="ExternalInput").ap()
    xn_b_d = nc.dram_tensor("xn_b", (HW - 128, BL, D), FP16,
                            kind="ExternalInput").ap()  # halves DMA'd separately
    # wfin[dp, dt, g, c, el] = lm_w[(g*EG+el)*C+c, dt*128+dp]
    wfin_d = nc.dram_tensor("wfin", (128, KT, NG, C, EG), FP16,
                            kind="ExternalInput").ap()
    # mask[b*EG+ep, c, e] = (e == ep); selects diagonal e-blocks
    mask_d = nc.dram_tensor("mask", (128, C, EG), FP16, kind="ExternalInput").ap()
    # sel3[b*EG+ep, bp] = (b == bp); partition-sums per batch
    sel3_d = nc.dram_tensor("sel3", (128, BL), FP16, kind="ExternalInput").ap()
    ident_d = nc.dram_tensor("ident", (128, 128), BF16, kind="ExternalInput").ap()
    preds_o = nc.dram_tensor("preds_o", (BL, C), FP32, kind="ExternalOutput").ap()

    from contextlib import ExitStack
    with tile.TileContext(nc) as tc, ExitStack() as _es:
        xt_p = _es.enter_context(tc.tile_pool(name="xt_p", bufs=1))
        agt_p = _es.enter_context(tc.tile_pool(name="agt_p", bufs=1))
        small_p = _es.enter_context(tc.tile_pool(name="small_p", bufs=1))
        st_p = _es.enter_context(tc.tile_pool(name="st_p", bufs=1))
        sn_p = _es.enter_context(tc.tile_pool(name="sn_p", bufs=1))
        xn_p = _es.enter_context(tc.tile_pool(name="xn_p", bufs=1))
        gt_p = _es.enter_context(tc.tile_pool(name="gt_p", bufs=1))
        wfin_p = _es.enter_context(tc.tile_pool(name="wfin_p", bufs=1))
        out_p = _es.enter_context(tc.tile_pool(name="out_p", bufs=1))
        m_p = _es.enter_context(tc.tile_pool(name="m_p", bufs=1))
        ps1 = _es.enter_context(tc.tile_pool(name="ps1", bufs=2, space="PSUM"))
        pst = _es.enter_context(tc.tile_pool(name="pst", bufs=2, space="PSUM"))
        ps2 = _es.enter_context(tc.tile_pool(name="ps2", bufs=3, space="PSUM"))
        psF = _es.enter_context(tc.tile_pool(name="psF", bufs=1, space="PSUM"))

        # ---- persistent tiles ----
        agt_hi_sb = agt_p.tile([128, KT, E], FP16, tag="agt_hi")
        agt8_sb = agt_p.tile([128, KT, 2, E], FP8, tag="agt8")
        agb_sb = small_p.tile([128, ET], FP32, tag="agb")
        xt_hi_sb = [xt_p.tile([128, KT, W1], FP16, tag=f"xth{n}",
                              name=f"xt_hi_sb{n}") for n in range(NCH)]
        xt8_sb = [xt_p.tile([128, 2, KT, W1], FP8, tag=f"xt8{n}",
                            name=f"xt8_sb{n}") for n in range(NCH)]
        xn_a_sb = [xn_p.tile([128, 4, D], FP16, tag=f"xn_a{h}",
                             name=f"xn_a_sb{h}") for h in range(2)]
        xn_b_sb = xn_p.tile([HW - 128, BL, D], FP16, tag="xn_b")
        wfin_sb = [wfin_p.tile([128, 3, NG, C, EG], FP16, tag=f"wfin{h}",
                               name=f"wfin_sb{h}") for h in range(2)]
        mask_sb = m_p.tile([128, C, EG], FP16, tag="mask")
        sel3_sb = m_p.tile([128, BL], FP16, tag="sel3")
        ident_sb = small_p.tile([128, 128], BF16, tag="ident")
        st_sb = [st_p.tile([128, T], BF16, tag=f"st{et}",
                           name=f"st_sb{et}") for et in range(ET)]
        sn_sb = [
            [sn_p.tile([128, E], FP16, tag=f"sn{b}_{ht}",
                       name=f"sn_sb{b}_{ht}") for ht in range(2)]
            for b in range(BL)
        ]
        gt_sb = [gt_p.tile([128, NG, BL, EG], FP16, tag=f"gt{dt}",
                           name=f"gt_sb{dt}")
                 for dt in range(KT)]

        # ---- DMA issue, by queue, in consumption order ----
        # mm1 chains consume xt by COLUMN chunk (nch), so xt streams in 4
        # column slices per tensor; chain n can start as soon as slice n
        # lands. xn gets a dedicated queue (gpsimd) so mm2 is never starved.
        # sync: mm1 hi path, then first wfin half
        # Queues are individually capped (~130 GB/s HW DGE, ~100 GB/s
        # gpsimd) and the fabric is globally byte/packet-limited, so the plan
        # balances bytes across all three queues with each queue's transfers
        # in strict consumption order. wfin is split 3 ways so each third
        # lands just before its finals consume it.
        nc.sync.dma_start(agt_hi_sb[:], agt_hi_d[:, :])
        nc.sync.dma_start(xt_hi_sb[0][:], xt_hi_d[:, 0])
        nc.sync.dma_start(agb_sb[:], agb_d[:, :])
        for n in range(1, NCH):
            nc.sync.dma_start(xt_hi_sb[n][:], xt_hi_d[:, n])
        nc.sync.dma_start(xn_a_sb[1][:], xn_a_d[:, 4:8])
        wfin_dma_s = nc.sync.dma_start(wfin_sb[0][:], wfin_d[:, 0:3])
        # scalar: mm1 fp8 path, xn upper half, wfin dt23
        nc.scalar.dma_start(agt8_sb[:], agt8_d[:, :, :, :])
        for n in range(NCH):
            nc.scalar.dma_start(xt8_sb[n][:, 0], xt8_d[:, n])
        nc.scalar.dma_start(xn_b_sb[:], xn_b_d[:, :, :])
        nc.scalar.dma_start(xn_a_sb[0][:], xn_a_d[:, 0:4])
        wfin_dma_a = nc.scalar.dma_start(wfin_sb[1][:], wfin_d[:, 3:6])
        nc.scalar.dma_start(mask_sb[:], mask_d[:, :, :])
        nc.scalar.dma_start(sel3_sb[:], sel3_d[:, :])
        # gpsimd: only non-critical loads — ident early, plus the FIRST wfin
        # pair which must land before the finals begin (~42us); gpsimd is
        # slow but has 30us of slack for it
        nc.gpsimd.dma_start(ident_sb[:], ident_d[:, :])

        # ---- PE warm-up: HAM boosts the PE clock (1.2 -> 2.4 GHz) only
        # after a few us of sustained matmul activity. Fill the DMA wait.
        warm_src = small_p.tile([128, W1], FP16, tag="warm_src")
        nc.vector.memset(warm_src[:], 0.0)
        warm_w = small_p.tile([128, 128], FP16, tag="warm_w")
        nc.vector.memset(warm_w[:], 0.0)
        for wi in range(NWARM):
            wps = ps1.tile([128, W1], FP32, tag="ps1", name=f"warm_ps{wi}")
            nc.tensor.matmul(wps[:], warm_w[:], warm_src[:], start=True,
                             stop=True)

        # ---- mm1: S^T[e,t] = (agt^T @ xt + b) > 0 ----
        th_insts = {}
        for nch in range(NCH):
            nc.vector.tensor_scalar_mul(xt8_sb[nch][:, 1], xt_hi_sb[nch][:],
                                        S_HI)
            for et in range(ET):
                esl = slice(et * 128, (et + 1) * 128)
                ps = ps1.tile([128, W1], FP32, tag="ps1",
                              name=f"ps1_{et}_{nch}")
                for kt in range(KT):
                    nc.tensor.matmul(
                        ps[:],
                        agt_hi_sb[:, kt, esl],
                        xt_hi_sb[nch][:, kt, :],
                        start=(kt == 0),
                        stop=False,
                    )
                for kt in range(KT):
                    nc.tensor.matmul(
                        ps[:],
                        agt8_sb[:, kt, :, esl],
                        xt8_sb[nch][:, :, kt, :],
                        start=False,
                        stop=(kt == KT - 1),
                        perf_mode=DR,
                    )
                th_insts[(et, nch)] = nc.vector.tensor_scalar(
                    st_sb[et][:, nch * W1:(nch + 1) * W1],
                    ps[:],
                    agb_sb[:, et:et + 1],
                    0.0,
                    mybir.AluOpType.add,
                    mybir.AluOpType.is_gt,
                )
            # transpose S^T -> S natural for the two batches of this chunk
            for b in (2 * nch, 2 * nch + 1):
                for ht in range(2):
                    w = 128 if ht == 0 else HW - 128
                    for et in range(ET):
                        pt = pst.tile([128, 128], BF16, tag="pst",
                                      name=f"pst_{b}_{ht}_{et}")
                        nc.tensor.transpose(
                            pt[0:w, :],
                            st_sb[et][:, b * HW + ht * 128:
                                      b * HW + ht * 128 + w],
                            ident_sb[:],
                        )
                        esl = slice(et * 128, (et + 1) * 128)
                        if (b + et) % 2 == 0:
                            nc.vector.tensor_copy(sn_sb[b][ht][0:w, esl],
                                                  pt[0:w, :])
                        else:
                            nc.scalar.copy(sn_sb[b][ht][0:w, esl],
                                           pt[0:w, :])

        # gate gpsimd's wfin pair so its packets stay off the fabric while
        # the xt stream feeds mm1
        add_dep_helper(wfin_dma_a.ins, th_insts[(0, 3)].ins,
                       reason="wfin after last mm1 chain")
        add_dep_helper(wfin_dma_s.ins, th_insts[(0, 3)].ins,
                       reason="wfin after last mm1 chain")

        # ---- mm2 (all dt), then final ----
        # All 96 final matmuls accumulate into ONE psum tile pf; the diagonal
        # e-mask commutes with the (dt, g) sum and is applied once at the end.
        # Running the finals strictly after mm2 relaxes the wfin DMA deadline
        # and every gt copy has long since completed.
        for dt in range(KT):
            dsl = slice(dt * 128, (dt + 1) * 128)
            for b in range(BL):
                pg = ps2.tile([128, E], FP32, tag="ps2", name=f"ps2_{b}_{dt}")
                nc.tensor.matmul(pg[:], xn_a_sb[b // 4][:, b % 4, dsl],
                                 sn_sb[b][0][:], start=True, stop=False)
                nc.tensor.matmul(pg[:], xn_b_sb[:, b, dsl],
                                 sn_sb[b][1][0:HW - 128, :],
                                 start=False, stop=True)
                if b % 2 == 0:
                    nc.vector.tensor_copy(gt_sb[dt][:, :, b, :], pg[:])
                else:
                    nc.scalar.copy(gt_sb[dt][:, :, b, :], pg[:])
        pf = psF.tile([128, C, EG], FP32, tag="pf")
        ip = 0
        for dt in range(KT):
            for g in range(NG):
                nc.tensor.matmul(
                    pf[:],
                    gt_sb[dt][:, g, :, :],
                    wfin_sb[dt // 3][:, dt % 3, g, :, :],
                    start=(ip == 0),
                    stop=(ip == KT * NG - 1),
                )
                ip += 1

        # ---- tail: mask diagonal, partition-sum per batch, reduce, scale ----
        msb = out_p.tile([128, C, EG], FP16, tag="msb")
        nc.vector.tensor_tensor(msb[:], pf[:], mask_sb[:],
                                mybir.AluOpType.mult)
        pf2 = ps1.tile([BL, C, EG], FP32, tag="ps1", name="pf2")
        nc.tensor.matmul(pf2[:], sel3_sb[:], msb[:], start=True, stop=True)
        red_sb = out_p.tile([BL, C], FP32, tag="red")
        nc.vector.tensor_reduce(red_sb[:], pf2[:], mybir.AxisListType.X,
                                mybir.AluOpType.add)
        out_sb = out_p.tile([BL, C], FP32, tag="out")
        nc.vector.tensor_scalar_mul(out_sb[:], red_sb[:], 1.0 / (HW * E))
        nc.sync.dma_start(preds_o[:, :], out_sb[:])

    return nc


_program_cache = {}

CONFIG = {"mm1": "fp16+fp8dr", "mm2": "fp16", "fin": "accum"}


def _get_program(**kw):
    key = tuple(sorted(kw.items()))
    if key not in _program_cache:
        _program_cache[key] = build_program()
    return _program_cache[key]


def make_in_maps(x, ag_w, ag_b, lm_w, cfg=None):
    import ml_dtypes

    E4 = ml_dtypes.float8_e4m3

    x = np.ascontiguousarray(np.asarray(x, dtype=np.float32))
    ag_w = np.asarray(ag_w, dtype=np.float32)
    ag_b = np.asarray(ag_b, dtype=np.float32)
    lm_w = np.asarray(lm_w, dtype=np.float32)

    agb = np.ascontiguousarray(ag_b.reshape(ET, 128).T)

    agt = np.ascontiguousarray(ag_w.T)               # (D, E)
    agt_hi16 = agt.astype(np.float16)
    agt_lo = agt - agt_hi16.astype(np.float32)
    agt_hi = np.ascontiguousarray(
        agt_hi16.reshape(KT, 128, E).transpose(1, 0, 2))
    agt8 = np.empty((128, KT, 2, E), dtype=E4)
    agt8[:, :, 0, :] = (agt_hi16.astype(np.float32) * S_HI) \
        .reshape(KT, 128, E).transpose(1, 0, 2).astype(E4)
    agt8[:, :, 1, :] = (agt_lo * S_LO) \
        .reshape(KT, 128, E).transpose(1, 0, 2).astype(E4)

    # wfin[dp, dt, g, c, el] = lm_w[(g*EG+el)*C+c, dt*128+dp]
    wfin = np.ascontiguousarray(
        lm_w.T.reshape(KT, 128, NG, EG, C)
        .transpose(1, 0, 2, 4, 3)
        .astype(np.float16)
    )
    ident = np.eye(128, dtype=ml_dtypes.bfloat16)
    ep = np.arange(128) % EG
    mask = np.ascontiguousarray(
        ((ep[:, None, None] == np.arange(EG)[None, None, :])
         * np.ones((128, C, EG))).astype(np.float16))
    bidx = np.arange(128) // EG
    sel3 = (bidx[:, None] == np.arange(BL)[None, :]).astype(np.float16)

    common = {"agb": agb, "agt_hi": agt_hi, "agt8": agt8, "wfin": wfin,
              "ident": ident, "mask": mask, "sel3": sel3}

    in_maps = []
    for i in range(NCORES):
        xs = x[i * BL:(i + 1) * BL].reshape(T, D)
        m = dict(common)
        xr = xs.reshape(BL, HW, D).astype(np.float16)
        m["xn_a"] = np.ascontiguousarray(xr[:, 0:128, :].transpose(1, 0, 2))
        m["xn_b"] = np.ascontiguousarray(xr[:, 128:HW, :].transpose(1, 0, 2))
        xt = np.ascontiguousarray(xs.T)              # (D, T)
        xt_hi16 = xt.astype(np.float16)
        xt_lo = xt - xt_hi16.astype(np.float32)
        m["xt_hi"] = np.ascontiguousarray(
            xt_hi16.reshape(KT, 128, NCH, W1).transpose(1, 2, 0, 3))
        m["xt8"] = np.ascontiguousarray(
            (xt_lo * S_LO)
            .reshape(KT, 128, NCH, W1).transpose(1, 2, 0, 3).astype(E4))
        in_maps.append(m)
    return in_maps


def kernel(x, ag_w, ag_b, lm_w):
    in_maps = make_in_maps(x, ag_w, ag_b, lm_w)
    nc = _get_program()
    res = run_bass_kernel_spmd(nc, in_maps, core_ids=list(range(NCORES)))
    preds = np.concatenate(
        [res.results[i]["preds_o"] for i in range(NCORES)], axis=0
    )
    return np.ascontiguousarray(preds.astype(np.float32))


# revision 28
# speedup vs baseline: 1.0907x; 1.0907x over previous
"""Trainium2 Bass kernel for nn_ConvexMLPBlock.

Reference computation (B=64, HW=196, D=768, E=256, C=10):
    S[b,h,e]  = (x[b,h,:] @ ag_w[e,:] + ag_b[e]) > 0          (sign patterns)
    z[b,h,p]  = x[b,h,:] @ lm_w[p,:]        (p = e*C + c)
    preds[b,c] = sum_{h,e} S[b,h,e] * z[b,h,e,c] / (HW*E)

Restructured to avoid materializing z (49 GFLOP -> ~10 GFLOP):
    G_b[e,d]   = sum_h S[b,h,e] * x[b,h,d]                    (per-batch masked moment)
    preds[b,c] = (1/(HW*E)) * sum_{e,d} G_b[e,d] * W[e,c,d]   (W = lm_w.reshape(E,C,D))

Sharding: data-parallel over B across the 8 NeuronCores (8 batches/core);
host concatenates the per-core (8,10) outputs.

Per-core pipeline:
    mm1: S^T[e,t] over all local tokens t=(b,h). Split x = x_hi(fp16) + x_lo;
         the hi*hi term runs as 6 fp16 matmuls (K=128 each); the two cross
         terms hi*lo + lo*hi run as 6 fp8 DoubleRow matmuls (each contracts
         K=128 over 2 pair slots: slot0 = w_hi8*x_lo8, slot1 = w_lo8*x_hi8,
         operands pre-scaled by 2^+-5.5 so each product has exact scale 1 and
         both fp8 operands sit in e4m3's normal range). Sign error: 14 flips
         out of 3.2M (CPU-simulated), rel err 2.3e-3 total.
    threshold: DVE tensor_scalar (psum + bias) > 0 -> 1.0/0.0 (bf16, exact)
    PE-transpose S^T -> S natural (h on partitions) per batch (bf16)
    mm2: G^T_b[d,e] contraction over h, fp16 (S exact in fp16; only x
         rounded). xn is zero-padded to 128 rows per (b, ht) piece so both
         pieces run K=128 (pad rows of sn are memset to 0 once).
    final: all 96 cross-product matmuls (dt, e-group) accumulate into ONE
           [128, C*EG] PSUM tile (the diagonal mask commutes with the sum);
           one DVE mask-mult + one selection matmul + reduce finish it.

DMA: ~17 large contiguous transfers (vs ~60 small) over the 3 DMA queues
(SP / Activation / gpsimd). The fabric is globally packet- and byte-limited
(~100 pkts/us, ~130 GB/s per queue), so each queue carries its transfers in
strict consumption order (xt -> xn -> wfin) and the late bulk loads (wfin)
are dep-gated behind mm1 thresholds to keep their packets off the fabric
while the xt stream feeds mm1. One SBUF tile per transfer (a tile with two
writers serializes them and makes every reader wait for both).
"""

import numpy as np

import concourse.bass as bass
import concourse.mybir as mybir
import concourse.tile as tile
from concourse.tile import add_dep_helper
from concourse.bass_utils import run_bass_kernel_spmd

# Problem constants (hardcoded per contract).
B = 64
HW = 196
D = 768
E = 256
C = 10
NCORES = 8
BL = B // NCORES          # local batches per core
T = BL * HW               # local tokens = 1568
KT = D // 128             # 6 d-tiles
ET = E // 128             # 2 e-tiles
W1 = 392                  # mm1 moving-dim chunk (4 chunks of 392 = 1568)
NCH = T // W1
EG = 16                   # e's per final-stage group
NG = E // EG              # 16 groups

FP32 = mybir.dt.float32
BF16 = mybir.dt.bfloat16
FP16 = mybir.dt.float16
FP8 = mybir.dt.float8e4

S_HI = 2.0 ** -5.5        # fp8 pre-scale on hi components
S_LO = 2.0 ** 5.5         # fp8 pre-scale on lo components

NWARM = 34                # PE warm-up matmuls bridging the DMA prologue


def _patched_drain_and_barrier(self, tick_clock, wait_clock):
    """This toolchain's walrus rejects >1 sync-wait on CTRL-class (Drain)
    instructions. Split the tail drain's global-clock waits across multiple
    single-wait drains. Semantics preserved: SP observes every DMA-queue
    semaphore before the all-engine barrier."""
    drain_inst = self.nc.sync.drain()
    wait_clock.add_sem_waits(
        drain_inst.ins, tile.ScopedClock({None: tick_clock.global_clock})
    )
    si = drain_inst.ins.sync_info
    if si is not None and si.on_wait is not None and len(si.on_wait) > 1:
        waits = list(si.on_wait)
        drain_inst.ins.sync_info = mybir.SyncInfo(
            on_wait=[waits[0]], on_update=list(si.on_update or [])
        )
        for w in waits[1:]:
            extra = self.nc.sync.drain()
            extra.ins.sync_info = mybir.SyncInfo(on_wait=[w], on_update=[])

    self.nc.all_engine_barrier()
    assert self.sems is not None
    popped = self.nc._tile_sem_poison_stack.pop()
    assert popped is self._sem_poison
    self.nc.clear_and_free_semaphores(list(self.sems.allocated().values()))
    self.nc.all_engine_barrier()


tile.TileContext._drain_and_barrier = _patched_drain_and_barrier


def _split_multiwait_json(bj: bytes) -> bytes:
    """Walrus in this toolchain accepts at most one sync-wait per instruction.
    For any instruction with N>1 waits, hoist N-1 waits onto same-engine NoOps
    inserted immediately before it. Engines execute program-order, so for
    compute instructions this is semantically identical; for DMAs it
    conservatively blocks the issuing engine instead of the queue."""
    import json

    m = json.loads(bj)
    changed = False
    for fn in m["functions"]:
        for bb in fn["blocks"]:
            new_insts = []
            for inst in bb["instructions"]:
                si = inst.get("sync_info")
                ow = (si or {}).get("on_wait") or []
                if len(ow) > 1:
                    for j, w in enumerate(ow[:-1]):
                        new_insts.append(
                            {
                                "name": f"{inst['name']}__w{j}",
                                "opcode": "NoOp",
                                "engine": inst["engine"],
                                "ins": [],
                                "outs": [],
                                "sync_info": {"on_update": [], "on_wait": [w]},
                            }
                        )
                    si["on_wait"] = [ow[-1]]
                    changed = True
                new_insts.append(inst)
            bb["instructions"] = new_insts
    if not changed:
        return bj
    return json.dumps(m).encode()


_orig_to_json_bytes = bass.Bass.to_json_bytes


def _patched_to_json_bytes(self, *a, **k):
    return _split_multiwait_json(_orig_to_json_bytes(self, *a, **k))


bass.Bass.to_json_bytes = _patched_to_json_bytes


def build_program():
    DR = mybir.MatmulPerfMode.DoubleRow

    nc = bass.Bass()

    # host layouts put the SBUF partition dim first so each load is ONE
    # contiguous-iteration DMA (dst/src element orders match).
    # xt layouts are nch-major so each column-chunk DMA is one contiguous
    # 4704B run per partition (descriptor generation cost scales with the
    # number of non-contiguous runs; short strided rows cost ~12us per issue)
    xt_hi_d = nc.dram_tensor("xt_hi", (128, NCH, KT, W1), FP16,
                             kind="ExternalInput").ap()
    xt8_d = nc.dram_tensor("xt8", (128, NCH, KT, 2, W1), FP8,
                           kind="ExternalInput").ap()
    agt_hi_d = nc.dram_tensor("agt_hi", (128, KT, E), FP16, kind="ExternalInput").ap()
    agt8_d = nc.dram_tensor("agt8", (128, KT, 2, E), FP8, kind="ExternalInput").ap()
    agb_d = nc.dram_tensor("agb", (128, ET), FP32, kind="ExternalInput").ap()
    # xn_a[r, b, d] = x[b, r, d]; xn_b[r, b, d] = x[b, 128+r, d]
    xn_a_d = nc.dram_tensor("xn_a", (128, BL, D), FP16, kind="ExternalInput").ap()
    xn_b_d = nc.dram_tensor("xn_b", (HW - 128, BL, D), FP16,
                            kind="ExternalInput").ap()  # halves DMA'd separately
    # wfin[dp, dt, g, c, el] = lm_w[(g*EG+el)*C+c, dt*128+dp]
    wfin_d = nc.dram_tensor("wfin", (128, KT, NG, C, EG), FP16,
                            kind="ExternalInput").ap()
    # mask[b*EG+ep, c, e] = (e == ep); selects diagonal e-blocks
    mask_d = nc.dram_tensor("mask", (128, C, EG), FP16, kind="ExternalInput").ap()
    # sel3[b*EG+ep, bp] = (b == bp); partition-sums per batch
    sel3_d = nc.dram_tensor("sel3", (128, BL), FP16, kind="ExternalInput").ap()
    ident_d = nc.dram_tensor("ident", (128, 128), BF16, kind="ExternalInput").ap()
    preds_o = nc.dram_tensor("preds_o", (BL, C), FP32, kind="ExternalOutput").ap()

    from contextlib import ExitStack
    with tile.TileContext(nc) as tc, ExitStack() as _es:
        xt_p = _es.enter_context(tc.tile_pool(name="xt_p", bufs=1))
        agt_p = _es.enter_context(tc.tile_pool(name="agt_p", bufs=1))
        small_p = _es.enter_context(tc.tile_pool(name="small_p", bufs=1))
        st_p = _es.enter_context(tc.tile_pool(name="st_p", bufs=1))
        sn_p = _es.enter_context(tc.tile_pool(name="sn_p", bufs=1))
        xn_p = _es.enter_context(tc.tile_pool(name="xn_p", bufs=1))
        gt_p = _es.enter_context(tc.tile_pool(name="gt_p", bufs=1))
        wfin_p = _es.enter_context(tc.tile_pool(name="wfin_p", bufs=1))
        out_p = _es.enter_context(tc.tile_pool(name="out_p", bufs=1))
        m_p = _es.enter_context(tc.tile_pool(name="m_p", bufs=1))
        ps1 = _es.enter_context(tc.tile_pool(name="ps1", bufs=2, space="PSUM"))
        pst = _es.enter_context(tc.tile_pool(name="pst", bufs=2, space="PSUM"))
        ps2 = _es.enter_context(tc.tile_pool(name="ps2", bufs=3, space="PSUM"))
        psF = _es.enter_context(tc.tile_pool(name="psF", bufs=1, space="PSUM"))

        # ---- persistent tiles ----
        agt_hi_sb = agt_p.tile([128, KT, E], FP16, tag="agt_hi")
        agt8_sb = agt_p.tile([128, KT, 2, E], FP8, tag="agt8")
        agb_sb = small_p.tile([128, ET], FP32, tag="agb")
        xt_hi_sb = [xt_p.tile([128, KT, W1], FP16, tag=f"xth{n}",
                              name=f"xt_hi_sb{n}") for n in range(NCH)]
        xt8_sb = [xt_p.tile([128, KT, 2, W1], FP8, tag=f"xt8{n}",
                            name=f"xt8_sb{n}") for n in range(NCH)]
        xn_a_sb = xn_p.tile([128, BL, D], FP16, tag="xn_a")
        xn_b_sb = xn_p.tile([HW - 128, BL, D], FP16, tag="xn_b")
        wfin_sb = [wfin_p.tile([128, 3, NG, C, EG], FP16, tag=f"wfin{h}",
                               name=f"wfin_sb{h}") for h in range(2)]
        mask_sb = m_p.tile([128, C, EG], FP16, tag="mask")
        sel3_sb = m_p.tile([128, BL], FP16, tag="sel3")
        ident_sb = small_p.tile([128, 128], BF16, tag="ident")
        st_sb = [st_p.tile([128, T], BF16, tag=f"st{et}",
                           name=f"st_sb{et}") for et in range(ET)]
        sn_sb = [
            [sn_p.tile([128, E], FP16, tag=f"sn{b}_{ht}",
                       name=f"sn_sb{b}_{ht}") for ht in range(2)]
            for b in range(BL)
        ]
        gt_sb = [gt_p.tile([128, NG, BL, EG], FP16, tag=f"gt{dt}",
                           name=f"gt_sb{dt}")
                 for dt in range(KT)]

        # ---- DMA issue, by queue, in consumption order ----
        # mm1 chains consume xt by COLUMN chunk (nch), so xt streams in 4
        # column slices per tensor; chain n can start as soon as slice n
        # lands. xn gets a dedicated queue (gpsimd) so mm2 is never starved.
        # sync: mm1 hi path, then first wfin half
        # Queues are individually capped (~130 GB/s HW DGE, ~100 GB/s
        # gpsimd) and the fabric is globally byte/packet-limited, so the plan
        # balances bytes across all three queues with each queue's transfers
        # in strict consumption order. wfin is split 3 ways so each third
        # lands just before its finals consume it.
        nc.sync.dma_start(agt_hi_sb[:], agt_hi_d[:, :])
        nc.sync.dma_start(xt_hi_sb[0][:], xt_hi_d[:, 0])
        nc.sync.dma_start(agb_sb[:], agb_d[:, :])
        for n in range(1, NCH):
            nc.sync.dma_start(xt_hi_sb[n][:], xt_hi_d[:, n])
        nc.sync.dma_start(xn_a_sb[:], xn_a_d[:, :, :])
        wfin_dma_s = nc.sync.dma_start(wfin_sb[0][:], wfin_d[:, 0:3])
        # scalar: mm1 fp8 path, xn upper half, wfin dt23
        nc.scalar.dma_start(agt8_sb[:], agt8_d[:, :, :, :])
        for n in range(NCH):
            nc.scalar.dma_start(xt8_sb[n][:], xt8_d[:, n])
        nc.scalar.dma_start(xn_b_sb[:], xn_b_d[:, :, :])
        wfin_dma_a = nc.scalar.dma_start(wfin_sb[1][:], wfin_d[:, 3:6])
        nc.scalar.dma_start(mask_sb[:], mask_d[:, :, :])
        nc.scalar.dma_start(sel3_sb[:], sel3_d[:, :])
        # gpsimd: only non-critical loads — ident early, plus the FIRST wfin
        # pair which must land before the finals begin (~42us); gpsimd is
        # slow but has 30us of slack for it
        nc.gpsimd.dma_start(ident_sb[:], ident_d[:, :])

        # ---- PE warm-up: HAM boosts the PE clock (1.2 -> 2.4 GHz) only
        # after a few us of sustained matmul activity. Fill the DMA wait.
        warm_src = small_p.tile([128, W1], FP16, tag="warm_src")
        nc.vector.memset(warm_src[:], 0.0)
        warm_w = small_p.tile([128, 128], FP16, tag="warm_w")
        nc.vector.memset(warm_w[:], 0.0)
        for wi in range(NWARM):
            wps = ps1.tile([128, W1], FP32, tag="ps1", name=f"warm_ps{wi}")
            nc.tensor.matmul(wps[:], warm_w[:], warm_src[:], start=True,
                             stop=True)

        # ---- mm1: S^T[e,t] = (agt^T @ xt + b) > 0 ----
        th_insts = {}
        for nch in range(NCH):
            for et in range(ET):
                esl = slice(et * 128, (et + 1) * 128)
                ps = ps1.tile([128, W1], FP32, tag="ps1",
                              name=f"ps1_{et}_{nch}")
                for kt in range(KT):
                    nc.tensor.matmul(
                        ps[:],
                        agt_hi_sb[:, kt, esl],
                        xt_hi_sb[nch][:, kt, :],
                        start=(kt == 0),
                        stop=False,
                    )
                for kt in range(KT):
                    nc.tensor.matmul(
                        ps[:],
                        agt8_sb[:, kt, :, esl],
                        xt8_sb[nch][:, kt, :, :],
                        start=False,
                        stop=(kt == KT - 1),
                        perf_mode=DR,
                    )
                th_insts[(et, nch)] = nc.vector.tensor_scalar(
                    st_sb[et][:, nch * W1:(nch + 1) * W1],
                    ps[:],
                    agb_sb[:, et:et + 1],
                    0.0,
                    mybir.AluOpType.add,
                    mybir.AluOpType.is_gt,
                )
            # transpose S^T -> S natural for the two batches of this chunk
            for b in (2 * nch, 2 * nch + 1):
                for ht in range(2):
                    w = 128 if ht == 0 else HW - 128
                    for et in range(ET):
                        pt = pst.tile([128, 128], BF16, tag="pst",
                                      name=f"pst_{b}_{ht}_{et}")
                        nc.tensor.transpose(
                            pt[0:w, :],
                            st_sb[et][:, b * HW + ht * 128:
                                      b * HW + ht * 128 + w],
                            ident_sb[:],
                        )
                        esl = slice(et * 128, (et + 1) * 128)
                        if (b + et) % 2 == 0:
                            nc.vector.tensor_copy(sn_sb[b][ht][0:w, esl],
                                                  pt[0:w, :])
                        else:
                            nc.scalar.copy(sn_sb[b][ht][0:w, esl],
                                           pt[0:w, :])

        # gate gpsimd's wfin pair so its packets stay off the fabric while
        # the xt stream feeds mm1
        add_dep_helper(wfin_dma_a.ins, th_insts[(0, 3)].ins,
                       reason="wfin after last mm1 chain")
        add_dep_helper(wfin_dma_s.ins, th_insts[(0, 3)].ins,
                       reason="wfin after last mm1 chain")

        # ---- mm2 (all dt), then final ----
        # All 96 final matmuls accumulate into ONE psum tile pf; the diagonal
        # e-mask commutes with the (dt, g) sum and is applied once at the end.
        # Running the finals strictly after mm2 relaxes the wfin DMA deadline
        # and every gt copy has long since completed.
        for dt in range(KT):
            dsl = slice(dt * 128, (dt + 1) * 128)
            for b in range(BL):
                pg = ps2.tile([128, E], FP32, tag="ps2", name=f"ps2_{b}_{dt}")
                nc.tensor.matmul(pg[:], xn_a_sb[:, b, dsl],
                                 sn_sb[b][0][:], start=True, stop=False)
                nc.tensor.matmul(pg[:], xn_b_sb[:, b, dsl],
                                 sn_sb[b][1][0:HW - 128, :],
                                 start=False, stop=True)
                if b % 2 == 0:
                    nc.vector.tensor_copy(gt_sb[dt][:, :, b, :], pg[:])
                else:
                    nc.scalar.copy(gt_sb[dt][:, :, b, :], pg[:])
        pf = psF.tile([128, C, EG], FP32, tag="pf")
        ip = 0
        for dt in range(KT):
            for g in range(NG):
                nc.tensor.matmul(
                    pf[:],
                    gt_sb[dt][:, g, :, :],
                    wfin_sb[dt // 3][:, dt % 3, g, :, :],
                    start=(ip == 0),
                    stop=(ip == KT * NG - 1),
                )
                ip += 1

        # ---- tail: mask diagonal, partition-sum per batch, reduce, scale ----
        msb = out_p.tile([128, C, EG], FP16, tag="msb")
        nc.vector.tensor_tensor(msb[:], pf[:], mask_sb[:],
                                mybir.AluOpType.mult)
        pf2 = ps1.tile([BL, C, EG], FP32, tag="ps1", name="pf2")
        nc.tensor.matmul(pf2[:], sel3_sb[:], msb[:], start=True, stop=True)
        red_sb = out_p.tile([BL, C], FP32, tag="red")
        nc.vector.tensor_reduce(red_sb[:], pf2[:], mybir.AxisListType.X,
                                mybir.AluOpType.add)
        out_sb = out_p.tile([BL, C], FP32, tag="out")
        nc.vector.tensor_scalar_mul(out_sb[:], red_sb[:], 1.0 / (HW * E))
        nc.sync.dma_start(preds_o[:, :], out_sb[:])

    return nc


_program_cache = {}

CONFIG = {"mm1": "fp16+fp8dr", "mm2": "fp16", "fin": "accum"}


def _get_program(**kw):
    key = tuple(sorted(kw.items()))
    if key not in _program_cache:
        _program_cache[key] = build_program()
    return _program_cache[key]


def make_in_maps(x, ag_w, ag_b, lm_w, cfg=None):
    import ml_dtypes

    E4 = ml_dtypes.float8_e4m3

    x = np.ascontiguousarray(np.asarray(x, dtype=np.float32))
    ag_w = np.asarray(ag_w, dtype=np.float32)
    ag_b = np.asarray(ag_b, dtype=np.float32)
    lm_w = np.asarray(lm_w, dtype=np.float32)

    agb = np.ascontiguousarray(ag_b.reshape(ET, 128).T)

    agt = np.ascontiguousarray(ag_w.T)               # (D, E)
    agt_hi16 = agt.astype(np.float16)
    agt_lo = agt - agt_hi16.astype(np.float32)
    agt_hi = np.ascontiguousarray(
        agt_hi16.reshape(KT, 128, E).transpose(1, 0, 2))
    agt8 = np.empty((128, KT, 2, E), dtype=E4)
    agt8[:, :, 0, :] = (agt_hi16.astype(np.float32) * S_HI) \
        .reshape(KT, 128, E).transpose(1, 0, 2).astype(E4)
    agt8[:, :, 1, :] = (agt_lo * S_LO) \
        .reshape(KT, 128, E).transpose(1, 0, 2).astype(E4)

    # wfin[dp, dt, g, c, el] = lm_w[(g*EG+el)*C+c, dt*128+dp]
    wfin = np.ascontiguousarray(
        lm_w.T.reshape(KT, 128, NG, EG, C)
        .transpose(1, 0, 2, 4, 3)
        .astype(np.float16)
    )
    ident = np.eye(128, dtype=ml_dtypes.bfloat16)
    ep = np.arange(128) % EG
    mask = np.ascontiguousarray(
        ((ep[:, None, None] == np.arange(EG)[None, None, :])
         * np.ones((128, C, EG))).astype(np.float16))
    bidx = np.arange(128) // EG
    sel3 = (bidx[:, None] == np.arange(BL)[None, :]).astype(np.float16)

    common = {"agb": agb, "agt_hi": agt_hi, "agt8": agt8, "wfin": wfin,
              "ident": ident, "mask": mask, "sel3": sel3}

    in_maps = []
    for i in range(NCORES):
        xs = x[i * BL:(i + 1) * BL].reshape(T, D)
        m = dict(common)
        xr = xs.reshape(BL, HW, D).astype(np.float16)
        m["xn_a"] = np.ascontiguousarray(xr[:, 0:128, :].transpose(1, 0, 2))
        m["xn_b"] = np.ascontiguousarray(xr[:, 128:HW, :].transpose(1, 0, 2))
        xt = np.ascontiguousarray(xs.T)              # (D, T)
        xt_hi16 = xt.astype(np.float16)
        xt_lo = xt - xt_hi16.astype(np.float32)
        m["xt_hi"] = np.ascontiguousarray(
            xt_hi16.reshape(KT, 128, NCH, W1).transpose(1, 2, 0, 3))
        xt8 = np.empty((128, NCH, KT, 2, W1), dtype=E4)
        xt8[:, :, :, 0, :] = (xt_lo * S_LO) \
            .reshape(KT, 128, NCH, W1).transpose(1, 2, 0, 3).astype(E4)
        xt8[:, :, :, 1, :] = (xt_hi16.astype(np.float32) * S_HI) \
            .reshape(KT, 128, NCH, W1).transpose(1, 2, 0, 3).astype(E4)
        m["xt8"] = xt8
        in_maps.append(m)
    return in_maps


def kernel(x, ag_w, ag_b, lm_w):
    in_maps = make_in_maps(x, ag_w, ag_b, lm_w)
    nc = _get_program()
    res = run_bass_kernel_spmd(nc, in_maps, core_ids=list(range(NCORES)))
    preds = np.concatenate(
        [res.results[i]["preds_o"] for i in range(NCORES)], axis=0
    )
    return np.ascontiguousarray(preds.astype(np.float32))
